# revision 4
# baseline (speedup 1.0000x reference)
"""Differential attention kernel for 8 Trainium2 NeuronCores — v6.

- v3 attention structure (bf16 q/k/scores/et/PV; fp8 there fails the
  2e-2 gate: peaky softmax rows expose e4m3 half-ulp directly).
- fp8e4 DoubleRow projections with both-side residual compensation:
  q = x8@W8 + dx8@W8 + x8@dW8, d-tile pairs contracted per DR matmul
  -> 0.75x the bf16 projection cost, residual error ~delta*delta.
- weights are scaled x64 before fp8 quantization (raw W ~ N(0, 0.022)
  sits in e4m3's subnormal range and its residual is unrepresentable);
  the 64 folds into the exp scale (scores x4096) and the v_aug ones
  column (64.0) so normalization cancels the v-side scale for free.
- exp emitted as one [128,1024] activation per skt pair.
"""

import math
import os
import time
from contextlib import ExitStack

import ml_dtypes
import numpy as np

import concourse.bass as bass
from concourse import bacc
import concourse.mybir as mybir
import concourse.tile as tile
from concourse.bass_utils import run_bass_kernel_spmd

B, S, D = 4, 4096, 2048
HD = 128
DV = 256
DVA = DV + 1      # + ones column for row sums
SQ = S // 2
N_CORES = 8
DEPTH = 12
WSCALE = 64.0     # weight pre-scale before fp8 quantization
SCALE = (HD ** -0.5) / (WSCALE * WSCALE)   # scores carry WSCALE^2

DT_P = D // 128   # 16 d-tiles
DP = DT_P // 2    # 8 d-tile pairs
SKT = S // 128    # 32 key tiles
SKP = SKT // 2    # 16 key tile pairs
SC = S // 512     # 8 s-chunks
QC = SQ // 512    # 4 q-chunks
SQT = SQ // 128   # 16 q tiles

BF16 = mybir.dt.bfloat16
F32 = mybir.dt.float32
F8 = mybir.dt.float8e4
FP8NP = ml_dtypes.float8_e4m3fn

INPUT_NAMES = ("x8", "xd8", "wq", "wqd", "wk", "wkd", "wv", "wvd", "lam")

_cache = {}


def build_nc():
    nc = bacc.Bacc("TRN2", target_bir_lowering=False, debug=False)

    # x fp8 + residual, packed as [128, dp, sc, pair(2), 512]
    x8_d = nc.declare_dram_parameter("x8", [128, DP, SC, 2, 512], F8, isOutput=False)
    xd8_d = nc.declare_dram_parameter("xd8", [128, DP, SC, 2, 512], F8, isOutput=False)
    # weights fp8 (+ residual), packed as [128, dp, pair(2), 256]
    w_names = ("wq", "wk", "wv")
    w_d = {n: nc.declare_dram_parameter(n, [128, DP, 2, DV], F8, isOutput=False)
           for n in w_names}
    wd_d = {n: nc.declare_dram_parameter(n + "d", [128, DP, 2, DV], F8, isOutput=False)
            for n in w_names}
    lam_d = nc.declare_dram_parameter("lam", [128, 1], F32, isOutput=False)
    out_d = nc.declare_dram_parameter("out", [SQ, DV], F32, isOutput=True)

    out = out_d.ap()

    with tile.TileContext(nc) as tc, ExitStack() as ctx:
        singles = ctx.enter_context(tc.tile_pool(name="singles", bufs=1))
        x_pool = ctx.enter_context(tc.tile_pool(name="x", bufs=20))
        xd_pool = ctx.enter_context(tc.tile_pool(name="xd", bufs=20))
        e_pool = ctx.enter_context(tc.tile_pool(name="e", bufs=6))
        o_pool = ctx.enter_context(tc.tile_pool(name="o", bufs=4))
        r_pool = ctx.enter_context(tc.tile_pool(name="r", bufs=8))

        # --- resident SBUF tensors -------------------------------------
        w_sb = {n: singles.tile([128, DP, 2, DV], F8, tag=f"w_{n}", name=f"w_{n}")
                for n in w_names}
        wd_sb = {n: singles.tile([128, DP, 2, DV], F8, tag=f"wd_{n}", name=f"wd_{n}")
                 for n in w_names}
        lam_sb = singles.tile([128, 1], F32, tag="lam")
        nc.sync.dma_start(out=lam_sb, in_=lam_d.ap())

        # chunk-0 x tiles interleaved with wk so the first k matmul can
        # start early; other weights follow.
        xts0, xdts0 = [], []
        for dp in range(DP):
            nc.sync.dma_start(out=w_sb["wk"][:, dp, :, :], in_=w_d["wk"].ap()[:, dp, :, :])
            nc.sync.dma_start(out=wd_sb["wk"][:, dp, :, :], in_=wd_d["wk"].ap()[:, dp, :, :])
            xt = x_pool.tile([128, 2, 512], F8, tag="xt", name=f"xt0_{dp}")
            nc.sync.dma_start(out=xt, in_=x8_d.ap()[:, dp, 0, :, :])
            xts0.append(xt)
            xdt = xd_pool.tile([128, 2, 512], F8, tag="xdt", name=f"xdt0_{dp}")
            nc.sync.dma_start(out=xdt, in_=xd8_d.ap()[:, dp, 0, :, :])
            xdts0.append(xdt)
        for n in ("wv", "wq"):
            for dp in range(DP):
                nc.sync.dma_start(out=w_sb[n][:, dp, :, :], in_=w_d[n].ap()[:, dp, :, :])
                nc.sync.dma_start(out=wd_sb[n][:, dp, :, :], in_=wd_d[n].ap()[:, dp, :, :])

        kT = singles.tile([128, 2, S], BF16, tag="kT")        # [dh, head, sk]
        qT = singles.tile([128, 2, SQ], BF16, tag="qT")       # [dh, head, sq]
        v_aug = singles.tile([128, SKT, DVA], BF16, tag="v")  # [s_row, s_tile, dv+1]
        pv1 = singles.tile([128, SQT, DVA], F32, tag="pv1")   # head-1 PV staging

        nc.vector.memset(v_aug[:, :, DV:DVA], WSCALE)         # scaled ones column

        # --- projections: one pass over the 8 s-chunks ------------------
        pctx = ExitStack()
        psum = pctx.enter_context(
            tc.tile_pool(name="psum_proj", bufs=4, space=bass.MemorySpace.PSUM)
        )

        # PE warm-up: junk matmuls fill the initial DMA wait so the PE is
        # ramped when the first real projection matmul issues.
        jt = singles.tile([128, 512], BF16, tag="junk")
        nc.vector.memset(jt, 0.0)
        jps = psum.tile([128, 512], F32, tag="big_ps", bufs=4, name="jps")
        for w in range(48):
            nc.tensor.matmul(jps, jt[:, 0:128], jt, start=True, stop=True)
        nc.vector.tensor_copy(jt, jps)

        DR = mybir.MatmulPerfMode.DoubleRow
        for sc in range(SC):
            if sc == 0:
                xts, xdts = xts0, xdts0
            else:
                xts, xdts = [], []
                for dp in range(DP):
                    xt = x_pool.tile([128, 2, 512], F8, tag="xt", name=f"xt{sc}_{dp}")
                    nc.sync.dma_start(out=xt, in_=x8_d.ap()[:, dp, sc, :, :])
                    xts.append(xt)
                    xdt = xd_pool.tile([128, 2, 512], F8, tag="xdt", name=f"xdt{sc}_{dp}")
                    nc.sync.dma_start(out=xdt, in_=xd8_d.ap()[:, dp, sc, :, :])
                    xdts.append(xdt)

            projs = [("wk", kT)] + ([("wq", qT)] if sc < QC else [])
            for wname, dst in projs:
                for h in range(2):
                    ps = psum.tile([128, 512], F32, tag="big_ps", bufs=4,
                                   name=f"ps{sc}{wname}{h}")
                    groups = [(w_sb[wname], xts), (w_sb[wname], xdts),
                              (wd_sb[wname], xts)]
                    for g, (wt, xl) in enumerate(groups):
                        for dp in range(DP):
                            nc.tensor.matmul(
                                ps,
                                wt[:, dp, :, h * HD:(h + 1) * HD],
                                xl[dp],
                                start=(g == 0 and dp == 0),
                                stop=(g == len(groups) - 1 and dp == DP - 1),
                                perf_mode=DR,
                            )
                    nc.vector.tensor_copy(dst[:, h, sc * 512:(sc + 1) * 512], ps)
            for i in range(4):
                vps = psum.tile([128, DV], F32, tag="v_ps", bufs=2,
                                name=f"vps{sc}_{i}")
                groups = [(xts, w_sb["wv"]), (xdts, w_sb["wv"]),
                          (xts, wd_sb["wv"])]
                for g, (xl, wt) in enumerate(groups):
                    for dp in range(DP):
                        nc.tensor.matmul(
                            vps,
                            xl[dp][:, :, i * 128:(i + 1) * 128],
                            wt[:, dp, :, :],
                            start=(g == 0 and dp == 0),
                            stop=(g == len(groups) - 1 and dp == DP - 1),
                            perf_mode=DR,
                        )
                nc.vector.tensor_copy(v_aug[:, sc * 4 + i, 0:DV], vps)

        pctx.close()

        # --- attention: head 1 then head 2 ------------------------------
        # scores for a skt pair land in one [128,1024] psum tile (2 banks),
        # exp'd in a single wide activation.
        psum = ctx.enter_context(
            tc.tile_pool(name="psum_att", bufs=2, space=bass.MemorySpace.PSUM)
        )
        psum_pv = ctx.enter_context(
            tc.tile_pool(name="psum_pv", bufs=4, space=bass.MemorySpace.PSUM)
        )
        for h in range(2):
            for qc in range(QC):
                pv_ps = [
                    psum_pv.tile([128, DVA], F32, tag="pv_ps", name=f"pv_ps{i}")
                    for i in range(4)
                ]
                for p in range(SKP):
                    sps = psum.tile([128, 1024], F32, tag="wide_ps", bufs=2,
                                    name=f"sps{p}")
                    for j in range(2):
                        skt = 2 * p + j
                        nc.tensor.matmul(
                            sps[:, j * 512:(j + 1) * 512],
                            kT[:, h, skt * 128:(skt + 1) * 128],
                            qT[:, h, qc * 512:(qc + 1) * 512],
                            start=True, stop=True,
                        )
                    et = e_pool.tile([128, 1024], BF16, tag="et", name=f"et{p}")
                    nc.scalar.activation(
                        out=et, in_=sps,
                        func=mybir.ActivationFunctionType.Exp,
                        scale=SCALE,
                    )
                    for j in range(2):
                        skt = 2 * p + j
                        for i in range(4):
                            nc.tensor.matmul(
                                pv_ps[i],
                                et[:, j * 512 + i * 128:j * 512 + (i + 1) * 128],
                                v_aug[:, skt, :],
                                start=(p == 0 and j == 0),
                                stop=(p == SKP - 1 and j == 1),
                            )
                for i in range(4):
                    idx = qc * 4 + i
                    if h == 0:
                        nc.vector.tensor_copy(pv1[:, idx, :], pv_ps[i])
                    else:
                        r1 = r_pool.tile([128, 1], F32, tag="r1", name=f"r1_{idx}")
                        r2 = r_pool.tile([128, 1], F32, tag="r2", name=f"r2_{idx}")
                        nc.vector.reciprocal(r1, pv1[:, idx, DV:DVA])
                        nc.vector.reciprocal(r2, pv_ps[i][:, DV:DVA])
                        r2l = r_pool.tile([128, 1], F32, tag="r2l", name=f"r2l_{idx}")
                        nc.vector.tensor_mul(r2l, r2, lam_sb)
                        o1 = o_pool.tile([128, DV], F32, tag="o1", name=f"o1_{idx}")
                        o2 = o_pool.tile([128, DV], F32, tag="o2", name=f"o2_{idx}")
                        nc.vector.tensor_scalar_mul(o1, pv1[:, idx, 0:DV], r1)
                        nc.vector.tensor_scalar_mul(o2, pv_ps[i][:, 0:DV], r2l)
                        ot = o_pool.tile([128, DV], F32, tag="ot", name=f"ot_{idx}")
                        nc.vector.tensor_sub(ot, o1, o2)
                        nc.sync.dma_start(
                            out=out[idx * 128:(idx + 1) * 128, :], in_=ot
                        )

    nc.compile()
    return nc


def _lam(lambda_q1, lambda_q2, lambda_k1, lambda_k2):
    lam_init = 0.8 - 0.6 * math.exp(-0.3 * DEPTH)
    l1 = math.exp(float(np.sum(lambda_q1.astype(np.float64) * lambda_k1.astype(np.float64))))
    l2 = math.exp(float(np.sum(lambda_q2.astype(np.float64) * lambda_k2.astype(np.float64))))
    return l1 + l2 + lam_init


def _pack_x(xT):
    """[D, S] f32 -> fp8 main + fp8 residual, packed [128, DP, SC, 2, 512]."""
    x8 = xT.astype(FP8NP)
    xd8 = (xT - x8.astype(np.float32)).astype(FP8NP)
    def pack(a):
        return np.ascontiguousarray(
            a.reshape(DP, 2, 128, SC, 512).transpose(2, 0, 3, 1, 4))
    return pack(x8), pack(xd8)


def _pack_w(W):
    """[D, 256] f32 -> x64-scaled fp8 main + residual, packed [128, DP, 2, 256]."""
    Ws = W * WSCALE
    w8 = Ws.astype(FP8NP)
    wd8 = (Ws - w8.astype(np.float32)).astype(FP8NP)
    def pack(a):
        return np.ascontiguousarray(
            a.reshape(DP, 2, 128, DV).transpose(2, 0, 1, 3))
    return pack(w8), pack(wd8)


def kernel(x, WQ, WK, WV, lambda_q1, lambda_q2, lambda_k1, lambda_k2):
    if "nc" not in _cache:
        _cache["nc"] = build_nc()
    nc = _cache["nc"]

    lam = np.full((128, 1), _lam(lambda_q1, lambda_q2, lambda_k1, lambda_k2), np.float32)
    wq8, wq8d = _pack_w(np.asarray(WQ, np.float32))
    wk8, wk8d = _pack_w(np.asarray(WK, np.float32))
    wv8, wv8d = _pack_w(np.asarray(WV, np.float32))

    in_maps = []
    for c in range(N_CORES):
        b, qs = c // 2, (c % 2) * SQ
        xb = x[b] if qs == 0 else np.concatenate([x[b, qs:], x[b, :qs]], axis=0)
        xT = np.ascontiguousarray(xb.T, dtype=np.float32)
        x8, xd8 = _pack_x(xT)
        in_maps.append({"x8": x8, "xd8": xd8,
                        "wq": wq8, "wqd": wq8d,
                        "wk": wk8, "wkd": wk8d,
                        "wv": wv8, "wvd": wv8d,
                        "lam": lam})

    kres = None
    for attempt in range(3):
        try:
            kres = run_bass_kernel_spmd(nc, in_maps, list(range(N_CORES)))
            break
        except (ModuleNotFoundError, ImportError):
            # BASS_TRACE requested but this axon build has no NTFF hook
            os.environ["BASS_NEVER_TRACE"] = "1"
        except Exception:
            if attempt == 2:
                raise
            time.sleep(5)
    if kres is None:
        kres = run_bass_kernel_spmd(nc, in_maps, list(range(N_CORES)))
    _cache["last_results"] = kres
    res = kres.results

    out = np.empty((B, S, DV), np.float32)
    for c in range(N_CORES):
        b, qs = c // 2, (c % 2) * SQ
        out[b, qs:qs + SQ] = res[c]["out"]
    return out


# revision 9
# speedup vs baseline: 1.2051x; 1.2051x over previous
"""Differential attention kernel for 8 Trainium2 NeuronCores — v6.

- v3 attention structure (bf16 q/k/scores/et/PV; fp8 there fails the
  2e-2 gate: peaky softmax rows expose e4m3 half-ulp directly).
- fp8e4 DoubleRow projections with both-side residual compensation:
  q = x8@W8 + dx8@W8 + x8@dW8, d-tile pairs contracted per DR matmul
  -> 0.75x the bf16 projection cost, residual error ~delta*delta.
- weights are scaled x64 before fp8 quantization (raw W ~ N(0, 0.022)
  sits in e4m3's subnormal range and its residual is unrepresentable);
  the 64 folds into the exp scale (scores x4096) and the v_aug ones
  column (64.0) so normalization cancels the v-side scale for free.
- exp emitted as one [128,1024] activation per skt pair.
"""

import math
import os
import time
from contextlib import ExitStack

import ml_dtypes
import numpy as np

import concourse.bass as bass
from concourse import bacc
import concourse.mybir as mybir
import concourse.tile as tile
from concourse.bass_utils import run_bass_kernel_spmd

B, S, D = 4, 4096, 2048
HD = 128
DV = 256
DVA = DV + 1      # + ones column for row sums
SQ = S // 2
N_CORES = 8
DEPTH = 12
WSCALE = 64.0     # weight pre-scale before fp8 quantization
SCALE = (HD ** -0.5) / (WSCALE * WSCALE)   # scores carry WSCALE^2

DT_P = D // 128   # 16 d-tiles
DP = DT_P // 2    # 8 d-tile pairs
SKT = S // 128    # 32 key tiles
SKP = SKT // 2    # 16 key tile pairs
SC = S // 512     # 8 s-chunks
QC = SQ // 512    # 4 q-chunks
SQT = SQ // 128   # 16 q tiles

BF16 = mybir.dt.bfloat16
F32 = mybir.dt.float32
F8 = mybir.dt.float8e4
FP8NP = ml_dtypes.float8_e4m3fn

INPUT_NAMES = ("x8", "xd8", "wq", "wqd", "wk", "wkd", "wv", "wvd", "lam")

_cache = {}


def build_nc():
    nc = bacc.Bacc("TRN2", target_bir_lowering=False, debug=False)

    # x fp8 + residual, packed as [128, dp, sc, pair(2), 512]
    x8_d = nc.declare_dram_parameter("x8", [128, DP, SC, 2, 512], F8, isOutput=False)
    xd8_d = nc.declare_dram_parameter("xd8", [128, DP, SC, 2, 512], F8, isOutput=False)
    # weights fp8 (+ residual), packed as [128, dp, pair(2), 256]
    w_names = ("wq", "wk", "wv")
    w_d = {n: nc.declare_dram_parameter(n, [128, DP, 2, DV], F8, isOutput=False)
           for n in w_names}
    wd_d = {n: nc.declare_dram_parameter(n + "d", [128, DP, 2, DV], F8, isOutput=False)
            for n in w_names}
    lam_d = nc.declare_dram_parameter("lam", [128, 1], F32, isOutput=False)
    out_d = nc.declare_dram_parameter("out", [SQ, DV], F32, isOutput=True)

    out = out_d.ap()

    with tile.TileContext(nc) as tc, ExitStack() as ctx:
        singles = ctx.enter_context(tc.tile_pool(name="singles", bufs=1))
        x_pool = ctx.enter_context(tc.tile_pool(name="x", bufs=3))
        xd_pool = ctx.enter_context(tc.tile_pool(name="xd", bufs=3))
        e_pool = ctx.enter_context(tc.tile_pool(name="e", bufs=6))
        o_pool = ctx.enter_context(tc.tile_pool(name="o", bufs=4))
        r_pool = ctx.enter_context(tc.tile_pool(name="r", bufs=8))

        # --- resident SBUF tensors -------------------------------------
        w_sb = {n: singles.tile([128, DP, 2, DV], F8, tag=f"w_{n}", name=f"w_{n}")
                for n in w_names}
        wd_sb = {n: singles.tile([128, DP, 2, DV], F8, tag=f"wd_{n}", name=f"wd_{n}")
                 for n in w_names}
        lam_sb = singles.tile([128, 1], F32, tag="lam")
        nc.sync.dma_start(out=lam_sb, in_=lam_d.ap())

        # wk + chunk-0 x first so the first k matmul can start early;
        # other weights follow, then residuals. One DMA per tensor/chunk.
        nc.sync.dma_start(out=w_sb["wk"], in_=w_d["wk"].ap())
        xt0 = x_pool.tile([128, DP, 2, 512], F8, tag="xt", name="xt0")
        nc.sync.dma_start(out=xt0, in_=x8_d.ap()[:, :, 0, :, :])
        xdt0 = xd_pool.tile([128, DP, 2, 512], F8, tag="xdt", name="xdt0")
        nc.sync.dma_start(out=xdt0, in_=xd8_d.ap()[:, :, 0, :, :])
        for n in ("wv", "wq"):
            nc.sync.dma_start(out=w_sb[n], in_=w_d[n].ap())
        for n in ("wk", "wv", "wq"):
            nc.sync.dma_start(out=wd_sb[n], in_=wd_d[n].ap())

        kT = singles.tile([128, 2, S], BF16, tag="kT")        # [dh, head, sk]
        qT = singles.tile([128, 2, SQ], BF16, tag="qT")       # [dh, head, sq]
        v_aug = singles.tile([128, SKT, DVA], BF16, tag="v")  # [s_row, s_tile, dv+1]
        pv1 = singles.tile([128, SQT, DVA], F32, tag="pv1")   # head-1 PV staging

        nc.vector.memset(v_aug[:, :, DV:DVA], WSCALE)         # scaled ones column

        # --- projections: one pass over the 8 s-chunks ------------------
        pctx = ExitStack()
        psum = pctx.enter_context(
            tc.tile_pool(name="psum_proj", bufs=4, space=bass.MemorySpace.PSUM)
        )

        # PE warm-up: junk matmuls fill the initial DMA wait so the PE is
        # ramped when the first real projection matmul issues.
        jt = singles.tile([128, 512], BF16, tag="junk")
        nc.vector.memset(jt, 0.0)
        jps = psum.tile([128, 512], F32, tag="big_ps", bufs=4, name="jps")
        for w in range(48):
            nc.tensor.matmul(jps, jt[:, 0:128], jt, start=True, stop=True)
        nc.vector.tensor_copy(jt, jps)

        DR = mybir.MatmulPerfMode.DoubleRow
        for sc in range(SC):
            if sc == 0:
                xt, xdt = xt0, xdt0
            else:
                xt = x_pool.tile([128, DP, 2, 512], F8, tag="xt", name=f"xt{sc}")
                nc.sync.dma_start(out=xt, in_=x8_d.ap()[:, :, sc, :, :])
                xdt = xd_pool.tile([128, DP, 2, 512], F8, tag="xdt", name=f"xdt{sc}")
                nc.sync.dma_start(out=xdt, in_=xd8_d.ap()[:, :, sc, :, :])

            projs = [("wk", kT)] + ([("wq", qT)] if sc < QC else [])
            for wname, dst in projs:
                for h in range(2):
                    ps = psum.tile([128, 512], F32, tag="big_ps", bufs=4,
                                   name=f"ps{sc}{wname}{h}")
                    groups = [(w_sb[wname], xt), (w_sb[wname], xdt),
                              (wd_sb[wname], xt)]
                    for g, (wt, xl) in enumerate(groups):
                        for dp in range(DP):
                            nc.tensor.matmul(
                                ps,
                                wt[:, dp, :, h * HD:(h + 1) * HD],
                                xl[:, dp, :, :],
                                start=(g == 0 and dp == 0),
                                stop=(g == len(groups) - 1 and dp == DP - 1),
                                perf_mode=DR,
                            )
                    nc.vector.tensor_copy(dst[:, h, sc * 512:(sc + 1) * 512], ps)
            for i in range(4):
                vps = psum.tile([128, DV], F32, tag="v_ps", bufs=2,
                                name=f"vps{sc}_{i}")
                groups = [(xt, w_sb["wv"]), (xdt, w_sb["wv"]),
                          (xt, wd_sb["wv"])]
                for g, (xl, wt) in enumerate(groups):
                    for dp in range(DP):
                        nc.tensor.matmul(
                            vps,
                            xl[:, dp, :, i * 128:(i + 1) * 128],
                            wt[:, dp, :, :],
                            start=(g == 0 and dp == 0),
                            stop=(g == len(groups) - 1 and dp == DP - 1),
                            perf_mode=DR,
                        )
                nc.vector.tensor_copy(v_aug[:, sc * 4 + i, 0:DV], vps)

        pctx.close()

        # --- attention: head 1 then head 2 ------------------------------
        # scores for a skt pair land in one [128,1024] psum tile (2 banks),
        # exp'd in a single wide activation.
        psum = ctx.enter_context(
            tc.tile_pool(name="psum_att", bufs=2, space=bass.MemorySpace.PSUM)
        )
        psum_pv = ctx.enter_context(
            tc.tile_pool(name="psum_pv", bufs=4, space=bass.MemorySpace.PSUM)
        )
        def emit_pv(pv_ps, et, p):
            for j in range(2):
                skt = 2 * p + j
                for i in range(4):
                    nc.tensor.matmul(
                        pv_ps[i],
                        et[:, j * 512 + i * 128:j * 512 + (i + 1) * 128],
                        v_aug[:, skt, :],
                        start=(p == 0 and j == 0),
                        stop=(p == SKP - 1 and j == 1),
                    )

        for h in range(2):
            for qc in range(QC):
                pv_ps = [
                    psum_pv.tile([128, DVA], F32, tag="pv_ps", name=f"pv_ps{i}")
                    for i in range(4)
                ]
                pending = None
                for p in range(SKP):
                    sps = psum.tile([128, 1024], F32, tag="wide_ps", bufs=2,
                                    name=f"sps{p}")
                    for j in range(2):
                        skt = 2 * p + j
                        nc.tensor.matmul(
                            sps[:, j * 512:(j + 1) * 512],
                            kT[:, h, skt * 128:(skt + 1) * 128],
                            qT[:, h, qc * 512:(qc + 1) * 512],
                            start=True, stop=True,
                        )
                    et = e_pool.tile([128, 1024], BF16, tag="et", name=f"et{p}")
                    nc.scalar.activation(
                        out=et, in_=sps,
                        func=mybir.ActivationFunctionType.Exp,
                        scale=SCALE,
                    )
                    if pending is not None:
                        emit_pv(pv_ps, *pending)
                    pending = (et, p)
                emit_pv(pv_ps, *pending)
                for i in range(4):
                    idx = qc * 4 + i
                    if h == 0:
                        nc.vector.tensor_copy(pv1[:, idx, :], pv_ps[i])
                    else:
                        r1 = r_pool.tile([128, 1], F32, tag="r1", name=f"r1_{idx}")
                        r2 = r_pool.tile([128, 1], F32, tag="r2", name=f"r2_{idx}")
                        nc.vector.reciprocal(r1, pv1[:, idx, DV:DVA])
                        nc.vector.reciprocal(r2, pv_ps[i][:, DV:DVA])
                        r2l = r_pool.tile([128, 1], F32, tag="r2l", name=f"r2l_{idx}")
                        nc.vector.tensor_mul(r2l, r2, lam_sb)
                        o1 = o_pool.tile([128, DV], F32, tag="o1", name=f"o1_{idx}")
                        o2 = o_pool.tile([128, DV], F32, tag="o2", name=f"o2_{idx}")
                        nc.vector.tensor_scalar_mul(o1, pv1[:, idx, 0:DV], r1)
                        nc.vector.tensor_scalar_mul(o2, pv_ps[i][:, 0:DV], r2l)
                        ot = o_pool.tile([128, DV], F32, tag="ot", name=f"ot_{idx}")
                        nc.vector.tensor_sub(ot, o1, o2)
                        nc.sync.dma_start(
                            out=out[idx * 128:(idx + 1) * 128, :], in_=ot
                        )

    nc.compile()
    return nc


def _lam(lambda_q1, lambda_q2, lambda_k1, lambda_k2):
    lam_init = 0.8 - 0.6 * math.exp(-0.3 * DEPTH)
    l1 = math.exp(float(np.sum(lambda_q1.astype(np.float64) * lambda_k1.astype(np.float64))))
    l2 = math.exp(float(np.sum(lambda_q2.astype(np.float64) * lambda_k2.astype(np.float64))))
    return l1 + l2 + lam_init


def _pack_x(xT):
    """[D, S] f32 -> fp8 main + fp8 residual, packed [128, DP, SC, 2, 512]."""
    x8 = xT.astype(FP8NP)
    xd8 = (xT - x8.astype(np.float32)).astype(FP8NP)
    def pack(a):
        return np.ascontiguousarray(
            a.reshape(DP, 2, 128, SC, 512).transpose(2, 0, 3, 1, 4))
    return pack(x8), pack(xd8)


def _pack_w(W):
    """[D, 256] f32 -> x64-scaled fp8 main + residual, packed [128, DP, 2, 256]."""
    Ws = W * WSCALE
    w8 = Ws.astype(FP8NP)
    wd8 = (Ws - w8.astype(np.float32)).astype(FP8NP)
    def pack(a):
        return np.ascontiguousarray(
            a.reshape(DP, 2, 128, DV).transpose(2, 0, 1, 3))
    return pack(w8), pack(wd8)


def kernel(x, WQ, WK, WV, lambda_q1, lambda_q2, lambda_k1, lambda_k2):
    if "nc" not in _cache:
        _cache["nc"] = build_nc()
    nc = _cache["nc"]

    lam = np.full((128, 1), _lam(lambda_q1, lambda_q2, lambda_k1, lambda_k2), np.float32)
    wq8, wq8d = _pack_w(np.asarray(WQ, np.float32))
    wk8, wk8d = _pack_w(np.asarray(WK, np.float32))
    wv8, wv8d = _pack_w(np.asarray(WV, np.float32))

    in_maps = []
    for c in range(N_CORES):
        b, qs = c // 2, (c % 2) * SQ
        xb = x[b] if qs == 0 else np.concatenate([x[b, qs:], x[b, :qs]], axis=0)
        xT = np.ascontiguousarray(xb.T, dtype=np.float32)
        x8, xd8 = _pack_x(xT)
        in_maps.append({"x8": x8, "xd8": xd8,
                        "wq": wq8, "wqd": wq8d,
                        "wk": wk8, "wkd": wk8d,
                        "wv": wv8, "wvd": wv8d,
                        "lam": lam})

    kres = None
    for attempt in range(3):
        try:
            kres = run_bass_kernel_spmd(nc, in_maps, list(range(N_CORES)))
            break
        except (ModuleNotFoundError, ImportError):
            # BASS_TRACE requested but this axon build has no NTFF hook
            os.environ["BASS_NEVER_TRACE"] = "1"
        except Exception:
            if attempt == 2:
                raise
            time.sleep(5)
    if kres is None:
        kres = run_bass_kernel_spmd(nc, in_maps, list(range(N_CORES)))
    _cache["last_results"] = kres
    res = kres.results

    out = np.empty((B, S, DV), np.float32)
    for c in range(N_CORES):
        b, qs = c // 2, (c % 2) * SQ
        out[b, qs:qs + SQ] = res[c]["out"]
    return out


# revision 26
# speedup vs baseline: 1.2813x; 1.0632x over previous
"""Differential attention kernel for 8 Trainium2 NeuronCores — v8.

- fp8e4 DoubleRow projections, both-side residual compensated (0.75x bf16),
  weights pre-scaled x64 (raw W sits in e4m3's subnormal range).
- combined-probability PV: p = e1 - c*e2 with c = lam*s1/s2 per query,
  computed once per q-chunk -> ONE bf16 PV matmul pass (half the PV cost).
  Row sums via N=1 ones-matmuls (q-partitioned), PE-transposed to rows via
  a host identity, lam folded into c, x64 v-scale folded into the ones vec.
- filler-queue emission: chunk 4-7 K/V projections and the previous
  q-chunk's combine+PV drain into the ACT(exp)-bound score stream so the
  PE never idles while the scalar engine works through ~133us of exp.
- single 8-bank PSUM budget: score/proj share one [128,1024] ring (4),
  sums (1) + transpose rows (1) + packed PV accumulators (2).
"""

import math
import os
import time
from collections import deque
from contextlib import ExitStack

import ml_dtypes
import numpy as np

import concourse.bass as bass
from concourse import bacc
import concourse.mybir as mybir
import concourse.tile as tile
from concourse.bass_utils import run_bass_kernel_spmd

B, S, D = 4, 4096, 2048
HD = 128
DV = 256
SQ = S // 2
N_CORES = 8
DEPTH = 12
WSCALE = 64.0     # weight pre-scale before fp8 quantization
SCALE = (HD ** -0.5) / (WSCALE * WSCALE)   # scores carry WSCALE^2

DT_P = D // 128   # 16 d-tiles
DP = DT_P // 2    # 8 d-tile pairs
SKT = S // 128    # 32 key tiles
SKP = SKT // 2    # 16 key tile pairs
SC = S // 512     # 8 s-chunks
QC = SQ // 512    # 4 q-chunks
SQT = SQ // 128   # 16 q tiles

BF16 = mybir.dt.bfloat16
F32 = mybir.dt.float32
F8 = mybir.dt.float8e4
FP8NP = ml_dtypes.float8_e4m3fn

INPUT_NAMES = ("x8", "xd8", "wq", "wqd", "wk", "wkd", "wv", "wvd", "lam", "ident")

_cache = {}
DRAIN_BUDGET = 100   # 0 = no interleave (debug)


def build_nc():
    nc = bacc.Bacc("TRN2", target_bir_lowering=False, debug=False)

    x8_d = nc.declare_dram_parameter("x8", [128, DP, SC, 2, 512], F8, isOutput=False)
    xd8_d = nc.declare_dram_parameter("xd8", [128, DP, SC, 2, 512], F8, isOutput=False)
    w_names = ("wq", "wk", "wv")
    w_d = {n: nc.declare_dram_parameter(n, [128, DP, 2, DV], F8, isOutput=False)
           for n in w_names}
    wd_d = {n: nc.declare_dram_parameter(n + "d", [128, DP, 2, DV], F8, isOutput=False)
            for n in w_names}
    lam_d = nc.declare_dram_parameter("lam", [128, 1], F32, isOutput=False)
    ident_d = nc.declare_dram_parameter("ident", [128, 128], BF16, isOutput=False)
    out_d = nc.declare_dram_parameter("out", [SQ, DV], F32, isOutput=True)

    out = out_d.ap()
    DR = mybir.MatmulPerfMode.DoubleRow

    with tile.TileContext(nc) as tc, ExitStack() as ctx:
        singles = ctx.enter_context(tc.tile_pool(name="singles", bufs=1))
        x_pool = ctx.enter_context(tc.tile_pool(name="x", bufs=2))
        xd_pool = ctx.enter_context(tc.tile_pool(name="xd", bufs=2))
        t_pool = ctx.enter_context(tc.tile_pool(name="t", bufs=2))
        c_pool = ctx.enter_context(tc.tile_pool(name="c", bufs=2))
        o_pool = ctx.enter_context(tc.tile_pool(name="o", bufs=4))
        r_pool = ctx.enter_context(tc.tile_pool(name="r", bufs=2))

        # --- resident SBUF tensors -------------------------------------
        w_sb = {n: singles.tile([128, DP, 2, DV], F8, tag=f"w_{n}", name=f"w_{n}")
                for n in w_names}
        wd_sb = {n: singles.tile([128, DP, 2, DV], F8, tag=f"wd_{n}", name=f"wd_{n}")
                 for n in w_names}
        lam_sb = singles.tile([128, 1], F32, tag="lam")
        id_sb = singles.tile([128, 128], BF16, tag="ident")
        nc.sync.dma_start(out=lam_sb, in_=lam_d.ap())
        nc.sync.dma_start(out=id_sb, in_=ident_d.ap())

        nc.sync.dma_start(out=w_sb["wk"], in_=w_d["wk"].ap())

        kT = singles.tile([128, 2, S], BF16, tag="kT")        # [dh, head, sk]
        qT = singles.tile([128, 2, SQ], BF16, tag="qT")       # [dh, head, sq]
        v_sb = singles.tile([128, SKT, DV], BF16, tag="v")    # [s_row, s_tile, dv]
        et_all = singles.tile([128, 2, SKP + 2, 1024], BF16, tag="et")  # per-qc exp tiles
        # pairs 0,1 alternate buffers by qc parity so the next q-chunk's exp
        # need not wait for this chunk's combine of the same pair
        et_slot = lambda qc, p: p if p >= 2 else (SKP + p if qc % 2 else p)
        ones_sb = singles.tile([128, 1], BF16, tag="ones")    # = WSCALE for sum matmuls
        sums_sb = singles.tile([128, 8], F32, tag="sums_sb")
        nc.vector.memset(ones_sb, WSCALE)

        # --- psum pools: 4 + 1 + 1 + 2 = 8 banks -----------------------
        psum_s = ctx.enter_context(
            tc.tile_pool(name="psum_s", bufs=2, space=bass.MemorySpace.PSUM)
        )
        psum_aux = ctx.enter_context(
            tc.tile_pool(name="psum_aux", bufs=1, space=bass.MemorySpace.PSUM)
        )
        psum_pv = ctx.enter_context(
            tc.tile_pool(name="psum_pv", bufs=1, space=bass.MemorySpace.PSUM)
        )

        def wide_ps(name):
            return psum_s.tile([128, 1024], F32, tag="wide_ps", bufs=2, name=name)

        # x DMA: one transfer per tensor per chunk
        x_tiles = {}

        def load_x(sc):
            if sc in x_tiles:
                return
            xt = x_pool.tile([128, DP, 2, 512], F8, tag="xt", name=f"xt{sc}")
            nc.sync.dma_start(out=xt, in_=x8_d.ap()[:, :, sc, :, :])
            xdt = xd_pool.tile([128, DP, 2, 512], F8, tag="xdt", name=f"xdt{sc}")
            nc.sync.dma_start(out=xdt, in_=xd8_d.ap()[:, :, sc, :, :])
            x_tiles[sc] = (xt, xdt)

        load_x(0)
        for n in ("wv", "wq"):
            nc.sync.dma_start(out=w_sb[n], in_=w_d[n].ap())
        for n in ("wk", "wv", "wq"):
            nc.sync.dma_start(out=wd_sb[n], in_=wd_d[n].ap())

        # PE warm-up over the initial DMA wait
        jt = singles.tile([128, 512], BF16, tag="junk")
        nc.vector.memset(jt, 0.0)
        jps = wide_ps("jps")
        for w in range(40):
            nc.tensor.matmul(jps[:, 0:512], jt[:, 0:128], jt, start=True, stop=True)
        nc.vector.tensor_copy(jt, jps[:, 0:512])

        # --- projection emitters (psum borrowed from the wide ring) ----
        def proj_kq(wname, dst, sc, h):
            xt, xdt = x_tiles[sc]
            ps = wide_ps(f"ps{sc}{wname}{h}")
            gl = [(w_sb[wname], xt), (w_sb[wname], xdt), (wd_sb[wname], xt)]
            for g, (wt, xl) in enumerate(gl):
                for dp in range(DP):
                    nc.tensor.matmul(
                        ps[:, 0:512], wt[:, dp, :, h * HD:(h + 1) * HD],
                        xl[:, dp, :, :],
                        start=(g == 0 and dp == 0),
                        stop=(g == 2 and dp == DP - 1),
                        perf_mode=DR,
                    )
            nc.vector.tensor_copy(dst[:, h, sc * 512:(sc + 1) * 512], ps[:, 0:512])

        def proj_v(sc, i):
            xt, xdt = x_tiles[sc]
            ps = wide_ps(f"vps{sc}_{i}")
            gl = [(xt, w_sb["wv"]), (xdt, w_sb["wv"]), (xt, wd_sb["wv"])]
            for g, (xl, wt) in enumerate(gl):
                for dp in range(DP):
                    nc.tensor.matmul(
                        ps[:, 0:256], xl[:, dp, :, i * 128:(i + 1) * 128],
                        wt[:, dp, :, :],
                        start=(g == 0 and dp == 0),
                        stop=(g == 2 and dp == DP - 1),
                        perf_mode=DR,
                    )
            nc.vector.tensor_copy(v_sb[:, sc * 4 + i, :], ps[:, 0:256])

        # chunks 0-3: K, Q, V emitted sequentially (PE/DMA-bound head)
        for sc in range(QC):
            load_x(sc)
            for h in range(2):
                proj_kq("wk", kT, sc, h)
            for h in range(2):
                proj_kq("wq", qT, sc, h)
            for i in range(4):
                proj_v(sc, i)

        # prefetch + queue chunks 4-7 as attention-stream filler
        filler = deque()
        emitted = set()
        for sc in range(QC, SC):
            load_x(sc)
            for h in range(2):
                filler.append((2600, ("K", sc, h),
                               lambda sc=sc, h=h: proj_kq("wk", kT, sc, h)))
            for i in range(4):
                filler.append((1300, ("V", sc, i),
                               lambda sc=sc, i=i: proj_v(sc, i)))

        def drain(budget_ns):
            while filler and budget_ns > 0:
                ns, key, fn = filler.popleft()
                fn()
                emitted.add(key)
                budget_ns -= ns

        def drain_until(key):
            while filler and key not in emitted:
                ns, k, fn = filler.popleft()
                fn()
                emitted.add(k)

        # --- attention ---------------------------------------------------
        def emit_sums(sums_ps, qc, h, p, first, last):
            pm = et_slot(qc, p)
            for j in range(2):
                for sub in range(4):
                    nc.tensor.matmul(
                        sums_ps[:, 2 * sub + h:2 * sub + h + 1],
                        et_all[:, h, pm, j * 512 + sub * 128:j * 512 + (sub + 1) * 128],
                        ones_sb,
                        start=(first and j == 0 and sub == 0),
                        stop=(last and j == 1 and sub == 3),
                        skip_group_check=True,
                    )

        def queue_phase_b(qc, sums_ps):
            # sums -> (s1,s2) rows -> c = lam*s1/s2 (bf16, partition-bcast)
            nc.vector.tensor_copy(sums_sb, sums_ps)
            # c = lam*s1/s2 computed q-partitioned (partition-base-0 ops),
            # then bf16 column transposes assemble the [1,512] row.
            rcp2 = r_pool.tile([128, 4], F32, tag="rcp2", name=f"rcp2{qc}")
            ccol = r_pool.tile([128, 4], F32, tag="ccol", name=f"ccol{qc}")
            cbf = r_pool.tile([128, 4], BF16, tag="cbf", name=f"cbf{qc}")
            r1s = r_pool.tile([128, 4], F32, tag="r1s", name=f"r1s{qc}")
            for sub in range(4):
                nc.vector.reciprocal(rcp2[:, sub:sub + 1],
                                     sums_sb[:, 2 * sub + 1:2 * sub + 2])
                nc.vector.tensor_mul(ccol[:, sub:sub + 1],
                                     sums_sb[:, 2 * sub:2 * sub + 1],
                                     rcp2[:, sub:sub + 1])
                nc.vector.reciprocal(r1s[:, sub:sub + 1],
                                     sums_sb[:, 2 * sub:2 * sub + 1])
            nc.vector.tensor_scalar_mul(cbf, ccol, lam_sb)
            c_full = c_pool.tile([128, 1024], BF16, tag="cfull", name=f"cfull{qc}")

            def reduce_unit():
                trn_ps = psum_aux.tile([1, 512], BF16, tag="trn", name=f"trn{qc}")
                for sub in range(4):
                    nc.tensor.matmul(
                        trn_ps[:, sub * 128:(sub + 1) * 128],
                        cbf[:, sub:sub + 1],
                        id_sb, is_transpose=True, start=True, stop=True,
                        skip_group_check=True,
                    )
                cb = r_pool.tile([1, 512], BF16, tag="cb", name=f"cb{qc}")
                nc.vector.tensor_copy(cb, trn_ps)
                nc.gpsimd.partition_broadcast(c_full[:, 0:512], cb)
                nc.gpsimd.partition_broadcast(c_full[:, 512:1024], cb)
            filler.append((250, ("red", qc), reduce_unit))

            pv_all = psum_pv.tile([128, 4, DV], F32, tag="pv", name=f"pv{qc}")
            filler.append((300, ("bm", qc), lambda: nc.vector.memset(pv_all, 0.0)))

            def combine_pair(p):
                pm = et_slot(qc, p)
                def fn():
                    tmp = t_pool.tile([128, 1024], BF16, tag="tmp", name=f"tmp{qc}_{p}")
                    nc.vector.tensor_mul(tmp, et_all[:, 1, pm, :], c_full)
                    nc.vector.tensor_sub(et_all[:, 1, pm, :], et_all[:, 0, pm, :], tmp)
                return fn

            def pv_pair(p):
                pm = et_slot(qc, p)
                def fn():
                    for j in range(2):
                        skt = 2 * p + j
                        for i in range(4):
                            nc.tensor.matmul(
                                pv_all[:, i, :],
                                et_all[:, 1, pm, j * 512 + i * 128:j * 512 + (i + 1) * 128],
                                v_sb[:, skt, :],
                                start=False,
                                stop=(p == SKP - 1 and j == 1 and i == 3),
                                skip_group_check=True,
                            )
                return fn
            # combine (DVE) leads its PV (PE) by 2 units so the vector engine
            # has the data ready when the PE reaches the matmuls.
            for p in range(SKP + 2):
                if p < SKP:
                    filler.append((150, ("bc", qc, p), combine_pair(p)))
                if p >= 2:
                    filler.append((900, ("pv", qc, p - 2), pv_pair(p - 2)))

            def finish():
                for i in range(4):
                    idx = qc * 4 + i
                    ot = o_pool.tile([128, DV], F32, tag="ot", name=f"ot_{idx}")
                    nc.vector.tensor_scalar_mul(ot, pv_all[:, i, :], r1s[:, i:i + 1])
                    nc.sync.dma_start(out=out[idx * 128:(idx + 1) * 128, :], in_=ot)
            filler.append((200, ("bf", qc), finish))

        for qc in range(QC):
            sums_ps = psum_aux.tile([128, 8], F32, tag="sums", name=f"sums{qc}")
            pend = deque()
            slots = ([(h, p) for p in (0, 1) for h in (0, 1)]
                     + [(h, p) for p in range(2, SKP) for h in (0, 1)])
            for si, (h, p) in enumerate(slots):
                    # read-before-write guards: K proj for this pair's chunk;
                    # prev q-chunk's combine (h0 reads et[0,p]) / PV (h1
                    # overwrites the combined et[1,p]). Pairs 0,1 are
                    # double-buffered and need no guard.
                    if p >= 8:
                        drain_until(("K", p // 2, h))
                    if qc > 0 and p >= 2:
                        drain_until(("bc" if h == 0 else "pv", qc - 1, p))
                    sps = wide_ps(f"sps{qc}_{h}_{p}")
                    for j in range(2):
                        skt = 2 * p + j
                        nc.tensor.matmul(
                            sps[:, j * 512:(j + 1) * 512],
                            kT[:, h, skt * 128:(skt + 1) * 128],
                            qT[:, h, qc * 512:(qc + 1) * 512],
                            start=True, stop=True,
                        )
                    nc.scalar.activation(
                        out=et_all[:, h, et_slot(qc, p), :], in_=sps,
                        func=mybir.ActivationFunctionType.Exp,
                        scale=SCALE,
                    )
                    pend.append((h, p))
                    if len(pend) > 2:
                        hh, pp = pend.popleft()
                        emit_sums(sums_ps, qc, hh, pp,
                                  first=(hh == 0 and pp == 0), last=False)
                    drain(DRAIN_BUDGET)
            while pend:
                hh, pp = pend.popleft()
                emit_sums(sums_ps, qc, hh, pp, first=(hh == 0 and pp == 0),
                          last=(hh == 1 and pp == SKP - 1))
            queue_phase_b(qc, sums_ps)
            if DRAIN_BUDGET == 0:
                drain(float("inf"))
        drain(float("inf"))

    nc.compile()
    return nc


def _lam(lambda_q1, lambda_q2, lambda_k1, lambda_k2):
    lam_init = 0.8 - 0.6 * math.exp(-0.3 * DEPTH)
    l1 = math.exp(float(np.sum(lambda_q1.astype(np.float64) * lambda_k1.astype(np.float64))))
    l2 = math.exp(float(np.sum(lambda_q2.astype(np.float64) * lambda_k2.astype(np.float64))))
    return l1 + l2 + lam_init


def _pack_x(xT):
    """[D, S] f32 -> fp8 main + fp8 residual, packed [128, DP, SC, 2, 512]."""
    x8 = xT.astype(FP8NP)
    xd8 = (xT - x8.astype(np.float32)).astype(FP8NP)
    def pack(a):
        return np.ascontiguousarray(
            a.reshape(DP, 2, 128, SC, 512).transpose(2, 0, 3, 1, 4))
    return pack(x8), pack(xd8)


def _pack_w(W):
    """[D, 256] f32 -> x64-scaled fp8 main + residual, packed [128, DP, 2, 256]."""
    Ws = W * WSCALE
    w8 = Ws.astype(FP8NP)
    wd8 = (Ws - w8.astype(np.float32)).astype(FP8NP)
    def pack(a):
        return np.ascontiguousarray(
            a.reshape(DP, 2, 128, DV).transpose(2, 0, 1, 3))
    return pack(w8), pack(wd8)


def kernel(x, WQ, WK, WV, lambda_q1, lambda_q2, lambda_k1, lambda_k2):
    if "nc" not in _cache:
        _cache["nc"] = build_nc()
    nc = _cache["nc"]

    lam = np.full((128, 1), _lam(lambda_q1, lambda_q2, lambda_k1, lambda_k2), np.float32)
    ident = np.eye(128).astype(ml_dtypes.bfloat16)
    wq8, wq8d = _pack_w(np.asarray(WQ, np.float32))
    wk8, wk8d = _pack_w(np.asarray(WK, np.float32))
    wv8, wv8d = _pack_w(np.asarray(WV, np.float32))

    in_maps = []
    for c in range(N_CORES):
        b, qs = c // 2, (c % 2) * SQ
        xb = x[b] if qs == 0 else np.concatenate([x[b, qs:], x[b, :qs]], axis=0)
        xT = np.ascontiguousarray(xb.T, dtype=np.float32)
        x8, xd8 = _pack_x(xT)
        in_maps.append({"x8": x8, "xd8": xd8,
                        "wq": wq8, "wqd": wq8d,
                        "wk": wk8, "wkd": wk8d,
                        "wv": wv8, "wvd": wv8d,
                        "lam": lam, "ident": ident})

    kres = None
    for attempt in range(3):
        try:
            kres = run_bass_kernel_spmd(nc, in_maps, list(range(N_CORES)))
            break
        except (ModuleNotFoundError, ImportError):
            os.environ["BASS_NEVER_TRACE"] = "1"
        except Exception:
            if attempt == 2:
                raise
            time.sleep(5)
    if kres is None:
        kres = run_bass_kernel_spmd(nc, in_maps, list(range(N_CORES)))
    _cache["last_results"] = kres
    res = kres.results

    out = np.empty((B, S, DV), np.float32)
    for c in range(N_CORES):
        b, qs = c // 2, (c % 2) * SQ
        out[b, qs:qs + SQ] = res[c]["out"]
    return out


# revision 32
# speedup vs baseline: 1.2890x; 1.0060x over previous
"""Differential attention kernel for 8 Trainium2 NeuronCores — v8.

- fp8e4 DoubleRow projections, both-side residual compensated (0.75x bf16),
  weights pre-scaled x64 (raw W sits in e4m3's subnormal range).
- combined-probability PV: p = e1 - c*e2 with c = lam*s1/s2 per query,
  computed once per q-chunk -> ONE bf16 PV matmul pass (half the PV cost).
  Row sums via N=1 ones-matmuls (q-partitioned), PE-transposed to rows via
  a host identity, lam folded into c, x64 v-scale folded into the ones vec.
- filler-queue emission: chunk 4-7 K/V projections and the previous
  q-chunk's combine+PV drain into the ACT(exp)-bound score stream so the
  PE never idles while the scalar engine works through ~133us of exp.
- single 8-bank PSUM budget: score/proj share one [128,1024] ring (4),
  sums (1) + transpose rows (1) + packed PV accumulators (2).
"""

import math
import os
import time
from collections import deque
from contextlib import ExitStack

import ml_dtypes
import numpy as np

import concourse.bass as bass
from concourse import bacc
import concourse.mybir as mybir
import concourse.tile as tile
from concourse.bass_utils import run_bass_kernel_spmd

B, S, D = 4, 4096, 2048
HD = 128
DV = 256
SQ = S // 2
N_CORES = 8
DEPTH = 12
WSCALE = 64.0     # weight pre-scale before fp8 quantization
SCALE = (HD ** -0.5) / (WSCALE * WSCALE)   # scores carry WSCALE^2

DT_P = D // 128   # 16 d-tiles
DP = DT_P // 2    # 8 d-tile pairs
SKT = S // 128    # 32 key tiles
SKP = SKT // 2    # 16 key tile pairs
SC = S // 512     # 8 s-chunks
QC = SQ // 512    # 4 q-chunks
SQT = SQ // 128   # 16 q tiles

BF16 = mybir.dt.bfloat16
F32 = mybir.dt.float32
F8 = mybir.dt.float8e4
FP8NP = ml_dtypes.float8_e4m3fn

INPUT_NAMES = ("x8", "xd8", "wq", "wqd", "wk", "wkd", "wv", "wvd", "lam", "ident")

_cache = {}
DRAIN_BUDGET = 100   # 0 = no interleave (debug)


def build_nc():
    nc = bacc.Bacc("TRN2", target_bir_lowering=False, debug=False)

    x8_d = nc.declare_dram_parameter("x8", [128, DP, SC, 2, 512], F8, isOutput=False)
    xd8_d = nc.declare_dram_parameter("xd8", [128, DP, SC, 2, 512], F8, isOutput=False)
    w_names = ("wq", "wk", "wv")
    w_d = {n: nc.declare_dram_parameter(n, [128, DP, 2, DV], F8, isOutput=False)
           for n in w_names}
    wd_d = {n: nc.declare_dram_parameter(n + "d", [128, DP, 2, DV], F8, isOutput=False)
            for n in w_names}
    lam_d = nc.declare_dram_parameter("lam", [128, 1], F32, isOutput=False)
    ident_d = nc.declare_dram_parameter("ident", [128, 128], BF16, isOutput=False)
    out_d = nc.declare_dram_parameter("out", [SQ, DV], F32, isOutput=True)

    out = out_d.ap()
    DR = mybir.MatmulPerfMode.DoubleRow

    with tile.TileContext(nc) as tc, ExitStack() as ctx:
        singles = ctx.enter_context(tc.tile_pool(name="singles", bufs=1))
        x_pool = ctx.enter_context(tc.tile_pool(name="x", bufs=3))
        xd_pool = ctx.enter_context(tc.tile_pool(name="xd", bufs=3))
        t_pool = ctx.enter_context(tc.tile_pool(name="t", bufs=2))
        c_pool = ctx.enter_context(tc.tile_pool(name="c", bufs=2))
        o_pool = ctx.enter_context(tc.tile_pool(name="o", bufs=4))
        r_pool = ctx.enter_context(tc.tile_pool(name="r", bufs=2))

        # --- resident SBUF tensors -------------------------------------
        w_sb = {n: singles.tile([128, DP, 2, DV], F8, tag=f"w_{n}", name=f"w_{n}")
                for n in w_names}
        wd_sb = {n: singles.tile([128, DP, 2, DV], F8, tag=f"wd_{n}", name=f"wd_{n}")
                 for n in w_names}
        lam_sb = singles.tile([128, 1], F32, tag="lam")
        id_sb = singles.tile([128, 128], BF16, tag="ident")
        nc.sync.dma_start(out=lam_sb, in_=lam_d.ap())
        nc.sync.dma_start(out=id_sb, in_=ident_d.ap())

        nc.sync.dma_start(out=w_sb["wk"], in_=w_d["wk"].ap())

        kT = singles.tile([128, 2, S], BF16, tag="kT")        # [dh, head, sk]
        qT = singles.tile([128, 2, SQ], BF16, tag="qT")       # [dh, head, sq]
        v_sb = singles.tile([128, SKT, DV], BF16, tag="v")    # [s_row, s_tile, dv]
        et_all = singles.tile([128, 2, SKP + 2, 1024], BF16, tag="et")  # per-qc exp tiles
        # pairs 0,1 alternate buffers by qc parity so the next q-chunk's exp
        # need not wait for this chunk's combine of the same pair
        et_slot = lambda qc, p: p if p >= 2 else (SKP + p if qc % 2 else p)
        ones_sb = singles.tile([128, 1], BF16, tag="ones")    # = WSCALE for sum matmuls
        sums_sb = singles.tile([128, 8], F32, tag="sums_sb")
        nc.vector.memset(ones_sb, WSCALE)

        # --- psum pools: 4 + 1 + 1 + 2 = 8 banks -----------------------
        psum_s = ctx.enter_context(
            tc.tile_pool(name="psum_s", bufs=2, space=bass.MemorySpace.PSUM)
        )
        psum_aux = ctx.enter_context(
            tc.tile_pool(name="psum_aux", bufs=1, space=bass.MemorySpace.PSUM)
        )
        psum_pv = ctx.enter_context(
            tc.tile_pool(name="psum_pv", bufs=1, space=bass.MemorySpace.PSUM)
        )

        def wide_ps(name):
            return psum_s.tile([128, 1024], F32, tag="wide_ps", bufs=2, name=name)

        # x DMA: one transfer per tensor per chunk
        x_tiles = {}

        def load_x(sc):
            if sc in x_tiles:
                return
            xt = x_pool.tile([128, DP, 2, 512], F8, tag="xt", name=f"xt{sc}")
            nc.sync.dma_start(out=xt, in_=x8_d.ap()[:, :, sc, :, :])
            xdt = xd_pool.tile([128, DP, 2, 512], F8, tag="xdt", name=f"xdt{sc}")
            nc.sync.dma_start(out=xdt, in_=xd8_d.ap()[:, :, sc, :, :])
            x_tiles[sc] = (xt, xdt)

        load_x(0)
        for n in ("wv", "wq"):
            nc.sync.dma_start(out=w_sb[n], in_=w_d[n].ap())
        for n in ("wk", "wv", "wq"):
            nc.sync.dma_start(out=wd_sb[n], in_=wd_d[n].ap())

        # PE warm-up over the initial DMA wait
        jt = singles.tile([128, 512], BF16, tag="junk")
        nc.vector.memset(jt, 0.0)
        jps = wide_ps("jps")
        for w in range(40):
            nc.tensor.matmul(jps[:, 0:512], jt[:, 0:128], jt, start=True, stop=True)
        nc.vector.tensor_copy(jt, jps[:, 0:512])

        # --- projection emitters (psum borrowed from the wide ring) ----
        def proj_kq(wname, dst, sc, h):
            xt, xdt = x_tiles[sc]
            ps = wide_ps(f"ps{sc}{wname}{h}")
            gl = [(w_sb[wname], xt), (w_sb[wname], xdt), (wd_sb[wname], xt)]
            for g, (wt, xl) in enumerate(gl):
                for dp in range(DP):
                    nc.tensor.matmul(
                        ps[:, 0:512], wt[:, dp, :, h * HD:(h + 1) * HD],
                        xl[:, dp, :, :],
                        start=(g == 0 and dp == 0),
                        stop=(g == 2 and dp == DP - 1),
                        perf_mode=DR,
                    )
            nc.vector.tensor_copy(dst[:, h, sc * 512:(sc + 1) * 512], ps[:, 0:512])

        def proj_v(sc, i):
            xt, xdt = x_tiles[sc]
            ps = wide_ps(f"vps{sc}_{i}")
            gl = [(xt, w_sb["wv"]), (xdt, w_sb["wv"]), (xt, wd_sb["wv"])]
            for g, (xl, wt) in enumerate(gl):
                for dp in range(DP):
                    nc.tensor.matmul(
                        ps[:, 0:256], xl[:, dp, :, i * 128:(i + 1) * 128],
                        wt[:, dp, :, :],
                        start=(g == 0 and dp == 0),
                        stop=(g == 2 and dp == DP - 1),
                        perf_mode=DR,
                    )
            nc.vector.tensor_copy(v_sb[:, sc * 4 + i, :], ps[:, 0:256])

        # chunks 0-3: K and Q emitted sequentially (the attention stream
        # needs full qT + early kT); their V projections join the filler so
        # the exp stream starts ~20us earlier.
        filler = deque()
        emitted = set()
        for sc in range(QC):
            load_x(sc)
            for h in range(2):
                proj_kq("wk", kT, sc, h)
            for h in range(2):
                proj_kq("wq", qT, sc, h)
            if sc == 0:
                for i in range(4):
                    proj_v(sc, i)
        for sc in range(1, QC):
            for i in range(4):
                filler.append((1300, ("V", sc, i),
                               lambda sc=sc, i=i: proj_v(sc, i)))

        # prefetch + queue chunks 4-7 as attention-stream filler
        for sc in range(QC, SC):
            load_x(sc)
            for h in range(2):
                filler.append((2600, ("K", sc, h),
                               lambda sc=sc, h=h: proj_kq("wk", kT, sc, h)))
            for i in range(4):
                filler.append((1300, ("V", sc, i),
                               lambda sc=sc, i=i: proj_v(sc, i)))

        def drain(budget_ns):
            while filler and budget_ns > 0:
                ns, key, fn = filler.popleft()
                fn()
                emitted.add(key)
                budget_ns -= ns

        def drain_until(key):
            while filler and key not in emitted:
                ns, k, fn = filler.popleft()
                fn()
                emitted.add(k)

        # --- attention ---------------------------------------------------
        def emit_sums(sums_ps, qc, h, p, first, last):
            pm = et_slot(qc, p)
            for j in range(2):
                for sub in range(4):
                    nc.tensor.matmul(
                        sums_ps[:, 2 * sub + h:2 * sub + h + 1],
                        et_all[:, h, pm, j * 512 + sub * 128:j * 512 + (sub + 1) * 128],
                        ones_sb,
                        start=(first and j == 0 and sub == 0),
                        stop=(last and j == 1 and sub == 3),
                        skip_group_check=True,
                    )

        def queue_phase_b(qc, sums_ps):
            # sums -> (s1,s2) rows -> c = lam*s1/s2 (bf16, partition-bcast)
            nc.vector.tensor_copy(sums_sb, sums_ps)
            # c = lam*s1/s2 computed q-partitioned (partition-base-0 ops),
            # then bf16 column transposes assemble the [1,512] row.
            rcp2 = r_pool.tile([128, 4], F32, tag="rcp2", name=f"rcp2{qc}")
            ccol = r_pool.tile([128, 4], F32, tag="ccol", name=f"ccol{qc}")
            cbf = r_pool.tile([128, 4], BF16, tag="cbf", name=f"cbf{qc}")
            r1s = r_pool.tile([128, 4], F32, tag="r1s", name=f"r1s{qc}")
            for sub in range(4):
                nc.vector.reciprocal(rcp2[:, sub:sub + 1],
                                     sums_sb[:, 2 * sub + 1:2 * sub + 2])
                nc.vector.tensor_mul(ccol[:, sub:sub + 1],
                                     sums_sb[:, 2 * sub:2 * sub + 1],
                                     rcp2[:, sub:sub + 1])
                nc.vector.reciprocal(r1s[:, sub:sub + 1],
                                     sums_sb[:, 2 * sub:2 * sub + 1])
            nc.vector.tensor_scalar_mul(cbf, ccol, lam_sb)
            c_full = c_pool.tile([128, 1024], BF16, tag="cfull", name=f"cfull{qc}")

            def reduce_unit():
                trn_ps = psum_aux.tile([1, 512], BF16, tag="trn", name=f"trn{qc}")
                for sub in range(4):
                    nc.tensor.matmul(
                        trn_ps[:, sub * 128:(sub + 1) * 128],
                        cbf[:, sub:sub + 1],
                        id_sb, is_transpose=True, start=True, stop=True,
                        skip_group_check=True,
                    )
                cb = r_pool.tile([1, 512], BF16, tag="cb", name=f"cb{qc}")
                nc.vector.tensor_copy(cb, trn_ps)
                nc.gpsimd.partition_broadcast(c_full[:, 0:512], cb)
                nc.gpsimd.partition_broadcast(c_full[:, 512:1024], cb)
            filler.append((250, ("red", qc), reduce_unit))

            pv_all = psum_pv.tile([128, 4, DV], F32, tag="pv", name=f"pv{qc}")
            filler.append((300, ("bm", qc), lambda: nc.vector.memset(pv_all, 0.0)))

            def combine_pair(p):
                pm = et_slot(qc, p)
                def fn():
                    tmp = t_pool.tile([128, 1024], BF16, tag="tmp", name=f"tmp{qc}_{p}")
                    nc.vector.tensor_mul(tmp, et_all[:, 1, pm, :], c_full)
                    nc.vector.tensor_sub(et_all[:, 1, pm, :], et_all[:, 0, pm, :], tmp)
                return fn

            def pv_pair(p):
                pm = et_slot(qc, p)
                def fn():
                    for j in range(2):
                        skt = 2 * p + j
                        for i in range(4):
                            nc.tensor.matmul(
                                pv_all[:, i, :],
                                et_all[:, 1, pm, j * 512 + i * 128:j * 512 + (i + 1) * 128],
                                v_sb[:, skt, :],
                                start=False,
                                stop=(p == SKP - 1 and j == 1 and i == 3),
                                skip_group_check=True,
                            )
                return fn
            # combine (DVE) leads its PV (PE) by 2 units so the vector engine
            # has the data ready when the PE reaches the matmuls.
            for p in range(SKP + 2):
                if p < SKP:
                    filler.append((150, ("bc", qc, p), combine_pair(p)))
                if p >= 2:
                    filler.append((900, ("pv", qc, p - 2), pv_pair(p - 2)))

            def finish():
                for i in range(4):
                    idx = qc * 4 + i
                    ot = o_pool.tile([128, DV], F32, tag="ot", name=f"ot_{idx}")
                    nc.vector.tensor_scalar_mul(ot, pv_all[:, i, :], r1s[:, i:i + 1])
                    nc.sync.dma_start(out=out[idx * 128:(idx + 1) * 128, :], in_=ot)
            filler.append((200, ("bf", qc), finish))

        for qc in range(QC):
            sums_ps = psum_aux.tile([128, 8], F32, tag="sums", name=f"sums{qc}")
            pend = deque()
            slots = ([(h, p) for p in (0, 1) for h in (0, 1)]
                     + [(h, p) for p in range(2, SKP) for h in (0, 1)])
            for si, (h, p) in enumerate(slots):
                    # read-before-write guards: K proj for this pair's chunk;
                    # prev q-chunk's combine (h0 reads et[0,p]) / PV (h1
                    # overwrites the combined et[1,p]). Pairs 0,1 are
                    # double-buffered and need no guard.
                    if p >= 8:
                        drain_until(("K", p // 2, h))
                    if qc > 0 and p >= 2:
                        drain_until(("bc" if h == 0 else "pv", qc - 1, p))
                    sps = wide_ps(f"sps{qc}_{h}_{p}")
                    for j in range(2):
                        skt = 2 * p + j
                        nc.tensor.matmul(
                            sps[:, j * 512:(j + 1) * 512],
                            kT[:, h, skt * 128:(skt + 1) * 128],
                            qT[:, h, qc * 512:(qc + 1) * 512],
                            start=True, stop=True,
                        )
                    nc.scalar.activation(
                        out=et_all[:, h, et_slot(qc, p), :], in_=sps,
                        func=mybir.ActivationFunctionType.Exp,
                        scale=SCALE,
                    )
                    pend.append((h, p))
                    if len(pend) > 2:
                        hh, pp = pend.popleft()
                        emit_sums(sums_ps, qc, hh, pp,
                                  first=(hh == 0 and pp == 0), last=False)
                    drain(DRAIN_BUDGET)
            while pend:
                hh, pp = pend.popleft()
                emit_sums(sums_ps, qc, hh, pp, first=(hh == 0 and pp == 0),
                          last=(hh == 1 and pp == SKP - 1))
            queue_phase_b(qc, sums_ps)
            if DRAIN_BUDGET == 0:
                drain(float("inf"))
        drain(float("inf"))

    nc.compile()
    return nc


def _lam(lambda_q1, lambda_q2, lambda_k1, lambda_k2):
    lam_init = 0.8 - 0.6 * math.exp(-0.3 * DEPTH)
    l1 = math.exp(float(np.sum(lambda_q1.astype(np.float64) * lambda_k1.astype(np.float64))))
    l2 = math.exp(float(np.sum(lambda_q2.astype(np.float64) * lambda_k2.astype(np.float64))))
    return l1 + l2 + lam_init


def _pack_x(xT):
    """[D, S] f32 -> fp8 main + fp8 residual, packed [128, DP, SC, 2, 512]."""
    x8 = xT.astype(FP8NP)
    xd8 = (xT - x8.astype(np.float32)).astype(FP8NP)
    def pack(a):
        return np.ascontiguousarray(
            a.reshape(DP, 2, 128, SC, 512).transpose(2, 0, 3, 1, 4))
    return pack(x8), pack(xd8)


def _pack_w(W):
    """[D, 256] f32 -> x64-scaled fp8 main + residual, packed [128, DP, 2, 256]."""
    Ws = W * WSCALE
    w8 = Ws.astype(FP8NP)
    wd8 = (Ws - w8.astype(np.float32)).astype(FP8NP)
    def pack(a):
        return np.ascontiguousarray(
            a.reshape(DP, 2, 128, DV).transpose(2, 0, 1, 3))
    return pack(w8), pack(wd8)


def kernel(x, WQ, WK, WV, lambda_q1, lambda_q2, lambda_k1, lambda_k2):
    if "nc" not in _cache:
        _cache["nc"] = build_nc()
    nc = _cache["nc"]

    lam = np.full((128, 1), _lam(lambda_q1, lambda_q2, lambda_k1, lambda_k2), np.float32)
    ident = np.eye(128).astype(ml_dtypes.bfloat16)
    wq8, wq8d = _pack_w(np.asarray(WQ, np.float32))
    wk8, wk8d = _pack_w(np.asarray(WK, np.float32))
    wv8, wv8d = _pack_w(np.asarray(WV, np.float32))

    in_maps = []
    for c in range(N_CORES):
        b, qs = c // 2, (c % 2) * SQ
        xb = x[b] if qs == 0 else np.concatenate([x[b, qs:], x[b, :qs]], axis=0)
        xT = np.ascontiguousarray(xb.T, dtype=np.float32)
        x8, xd8 = _pack_x(xT)
        in_maps.append({"x8": x8, "xd8": xd8,
                        "wq": wq8, "wqd": wq8d,
                        "wk": wk8, "wkd": wk8d,
                        "wv": wv8, "wvd": wv8d,
                        "lam": lam, "ident": ident})

    kres = None
    for attempt in range(3):
        try:
            kres = run_bass_kernel_spmd(nc, in_maps, list(range(N_CORES)))
            break
        except (ModuleNotFoundError, ImportError):
            os.environ["BASS_NEVER_TRACE"] = "1"
        except Exception:
            if attempt == 2:
                raise
            time.sleep(5)
    if kres is None:
        kres = run_bass_kernel_spmd(nc, in_maps, list(range(N_CORES)))
    _cache["last_results"] = kres
    res = kres.results

    out = np.empty((B, S, DV), np.float32)
    for c in range(N_CORES):
        b, qs = c // 2, (c % 2) * SQ
        out[b, qs:qs + SQ] = res[c]["out"]
    return out
